# revision 1
# baseline (speedup 1.0000x reference)
"""Gaussian-weighted GNN message passing on 8 Trainium2 NeuronCores (v4).

out[b,i,f] = sum_{e: row_e=i} softmax_row(w)_e * X[b, col_e, f]
w_e = -0.5 * sum_d (u_val[e,d]-mu[d])^2 / (sigma[d]^2+eps)

Strategy (one SPMD program, 8 cores):
- Host: sort edges by destination row, shard rows (6250/core) + incident
  edges across cores; compute per-edge softmax weights sm_e in float64
  (exact segment softmax) alongside the sort; lay edges out in 128-slot
  tiles grouped by 128-row block. Replicate X (as [N, B*F] bf16) per core.
- Device, per 128-row block: TWO bulk dma_gather calls (int16 indices are
  limited to 32767, so cols are split at 32768 with a row-biased source AP)
  fetch all the block's source rows into one 3D SBUF tile [128, nt, 256]
  bf16. The calls rotate over all 4 SWDGE queues: each SDMA engine
  round-robins rings per packet, so 4 queues give ~4 outstanding HBM reads
  per engine — the gather is read-latency-bound, and this measured ~2.1x
  faster than one queue. Per-core exact edge counts ride in a register
  (num_idxs_reg) with trailing -1 indices so pad slots move no bytes.
  Per 128-edge tile: build
  S[p,r] = sm_p * (iota[p,r]==rloc_p) with one DVE tensor_scalar;
  accumulate pout += S^T @ Xg in PSUM via one bf16 matmul (f32
  accumulate). Copy PSUM->SBUF on the scalar engine, write out via HWDGE.
"""
import numpy as np

B, N, F, E, D = 2, 50000, 128, 800000, 4
BF = B * F
M = 8            # cores
NS = N // M      # rows per core
P = 128          # partitions / tile edge count
NBLK = (NS + P - 1) // P  # 49 row blocks per core
SPLIT = 32768    # int16 gather-index boundary

_cache = {}


def _build(T, toff, ntl, repeat=1):
    from concourse import bacc, mybir
    from concourse.tile import TileContext

    nt_lo, nt_hi, sk_lo, sk_hi = ntl
    f32 = mybir.dt.float32
    bf16 = mybir.dt.bfloat16
    i16 = mybir.dt.int16
    ntmax = int(max(int(a) + int(b) for a, b in zip(nt_lo, nt_hi)))
    nc = bacc.Bacc("TRN2", target_bir_lowering=False, debug=False, num_devices=M,
                   num_swdge_queues=4)
    xr = nc.dram_tensor("xr", [N, BF], bf16, kind="ExternalInput").ap()
    rloc = nc.dram_tensor("rloc", [P, T], f32, kind="ExternalInput").ap()
    smw = nc.dram_tensor("smw", [P, T], f32, kind="ExternalInput").ap()
    idxd = nc.dram_tensor("idxd", [P, 8 * T], i16, kind="ExternalInput").ap()
    cntd = nc.dram_tensor("cntd", [P, 2 * NBLK], mybir.dt.int32,
                          kind="ExternalInput").ap()
    iotaf = nc.dram_tensor("iotaf", [P, P], bf16, kind="ExternalInput").ap()
    out = nc.dram_tensor("out", [NS, BF], f32, kind="ExternalOutput").ap()

    with TileContext(nc) as tc:
        with (
            tc.tile_pool(name="const", bufs=1) as cpool,
            tc.tile_pool(name="g", bufs=10) as gpool,
            tc.tile_pool(name="s", bufs=8) as spool,
            tc.tile_pool(name="po", bufs=4, space="PSUM") as ppool,
            tc.tile_pool(name="o", bufs=4) as opool,
        ):
            rloc_t = cpool.tile([P, T], f32, tag="rloc")
            nc.sync.dma_start(out=rloc_t[:], in_=rloc[:])
            smw_t = cpool.tile([P, T], f32, tag="smw")
            nc.sync.dma_start(out=smw_t[:], in_=smw[:])
            idx_t = cpool.tile([P, 8 * T], i16, tag="idx")
            nc.sync.dma_start(out=idx_t[:], in_=idxd[:])
            cnt_t = cpool.tile([P, 2 * NBLK], mybir.dt.int32, tag="cnt")
            nc.sync.dma_start(out=cnt_t[:], in_=cntd[:])
            iota_t = cpool.tile([P, P], bf16, tag="iota")
            nc.sync.dma_start(out=iota_t[:], in_=iotaf[:])
            gcnt = nc.gpsimd.alloc_register("gcnt")
            qctr = [0]
            LOOK = 8

            def block_loop(_iv=None):
                gq = {}

                def alloc_and_clear(i):
                    if i >= NBLK:
                        return
                    gt = gpool.tile([P, ntmax, BF], bf16, tag="g", name="g")
                    for half, off, n_t in (
                        (0, 0, int(nt_lo[i])),
                        (1, int(nt_lo[i]), int(nt_hi[i])),
                    ):
                        sk = int((sk_lo, sk_hi)[half][i])
                        if sk < n_t:
                            nc.vector.memset(gt[:, off + sk : off + n_t, :], 0.0)
                    gq[i] = gt

                for i in range(LOOK + 1):
                    alloc_and_clear(i)
                for b in range(NBLK):
                    if b > 0:
                        alloc_and_clear(b + LOOK)
                    nl = int(nt_lo[b])
                    nh = int(nt_hi[b])
                    nt = nl + nh
                    t0 = int(toff[b])
                    r0 = b * P
                    rows_here = min(P, NS - r0)
                    g = gq.pop(b)
                    for half, off, n_t, src_ap in (
                        (0, 0, nl, xr[:SPLIT, :]), (1, nl, nh, xr[SPLIT:, :]),
                    ):
                        if not n_t:
                            continue
                        nc.gpsimd.reg_load(
                            gcnt, cnt_t[0:1, 2 * b + half : 2 * b + half + 1])
                        nc.gpsimd.dma_gather(
                            out_ap=g[:, off : off + n_t, :], in_ap=src_ap,
                            idxs_ap=idx_t[:, 8 * (t0 + off) : 8 * (t0 + off + n_t)],
                            num_idxs=n_t * P, num_idxs_reg=gcnt,
                            elem_size=BF, single_packet=False,
                            queue_num=qctr[0] % 4,
                        )
                        qctr[0] += 1
                    pout = ppool.tile([P, BF], f32, tag="pout", space="PSUM")
                    for k in range(nt):
                        t = t0 + k
                        S = spool.tile([P, P], bf16, tag="S")
                        nc.vector.tensor_scalar(
                            out=S[:], in0=iota_t[:],
                            scalar1=rloc_t[:, t : t + 1],
                            scalar2=smw_t[:, t : t + 1],
                            op0=mybir.AluOpType.is_equal,
                            op1=mybir.AluOpType.mult,
                        )
                        nc.tensor.matmul(
                            out=pout[:], lhsT=S[:], rhs=g[:, k, :],
                            start=(k == 0), stop=(k == nt - 1))
                    osb = opool.tile([P, BF], f32, tag="osb")
                    nc.scalar.activation(
                        out=osb[:], in_=pout[:],
                        func=mybir.ActivationFunctionType.Copy,
                    )
                    nc.sync.dma_start(out=out[r0 : r0 + rows_here, :],
                                      in_=osb[:rows_here, :])

            if repeat == 1:
                block_loop()
            else:
                with tc.For_i(0, repeat, 1) as _i:
                    block_loop(_i)

    nc.compile()
    return nc


def _prep(X, u_val, u_rows, u_cols, mu, sigma):
    """Host-side shard/sort/pad + exact softmax weights.

    Returns (T, toff, (nt_lo, nt_hi), in_maps)."""
    import ml_dtypes
    bf16 = ml_dtypes.bfloat16

    rows = np.asarray(u_rows).astype(np.int64)
    u = np.asarray(u_val).astype(np.float64)
    muf = np.asarray(mu)[0].astype(np.float64)
    sgf = np.asarray(sigma)[0].astype(np.float64)
    w = -0.5 * np.sum((u - muf) ** 2 / (sgf**2 + 1e-14), axis=1)
    z = np.exp(w)  # float64: no underflow for this data distribution
    ssum = np.bincount(rows, weights=z, minlength=N)
    sm = (z / ssum[rows]).astype(np.float32)

    perm = np.argsort(rows, kind="stable")
    rows_s = rows[perm]
    cols_s = np.asarray(u_cols)[perm].astype(np.int32)
    sm_s = sm[perm]

    # per (core, block, lo/hi) edge counts; block = (row % NS) // P
    core_of = rows_s // NS
    blk_of = (rows_s % NS) // P
    is_hi = (cols_s >= SPLIT).astype(np.int64)
    cnt = np.zeros((2, M, NBLK), dtype=np.int64)
    np.add.at(cnt, (is_hi, core_of, blk_of), 1)
    nt_lo = np.maximum(1, (cnt[0].max(axis=0) + P - 1) // P).astype(np.int64)
    nt_hi = np.maximum(1, (cnt[1].max(axis=0) + P - 1) // P).astype(np.int64)
    sk_lo = (cnt[0].min(axis=0) // P).astype(np.int64)
    sk_hi = (cnt[1].min(axis=0) // P).astype(np.int64)
    ntiles = nt_lo + nt_hi
    toff = np.concatenate([[0], np.cumsum(ntiles)]).astype(np.int64)
    T = int(toff[-1])

    iotaf = np.tile(np.arange(P, dtype=np.float32)[None, :], (P, 1)).astype(bf16)
    Xr = np.ascontiguousarray(
        np.asarray(X).transpose(1, 0, 2).reshape(N, BF)).astype(bf16)

    in_maps = []
    for c in range(M):
        sm_pad = np.zeros((T * P,), dtype=np.float32)
        rl_pad = np.full((T * P,), 999.0, dtype=np.float32)
        ix_pad = np.full((T * P,), -1, dtype=np.int16)
        cnts = np.ones((2 * NBLK,), dtype=np.int32)
        for b in range(NBLK):
            lo = np.searchsorted(rows_s, c * NS + b * P)
            hi = np.searchsorted(rows_s, min(c * NS + (b + 1) * P, (c + 1) * NS))
            if hi == lo:
                continue
            e_cols = cols_s[lo:hi]
            e_sm = sm_s[lo:hi]
            e_rl = (rows_s[lo:hi] - c * NS - b * P).astype(np.float32)
            mlo = e_cols < SPLIT
            for half, mask, bias, s0 in (
                (0, mlo, 0, toff[b] * P),
                (1, ~mlo, SPLIT, (toff[b] + nt_lo[b]) * P),
            ):
                n_e = int(mask.sum())
                if n_e == 0:
                    continue
                sm_pad[s0 : s0 + n_e] = e_sm[mask]
                rl_pad[s0 : s0 + n_e] = e_rl[mask]
                ix_pad[s0 : s0 + n_e] = (e_cols[mask] - bias).astype(np.int16)
                cnts[2 * b + half] = n_e
        for b in range(NBLK):
            for half, s0 in ((0, toff[b] * P), (1, (toff[b] + nt_lo[b]) * P)):
                if cnts[2 * b + half] == 1 and ix_pad[s0] < 0:
                    ix_pad[s0] = 0
        # slot (t, p) -> flat t*P + p ; [P, T] layouts = transpose;
        # gather-index wrap: flat i -> (i%16, i//16), replicated x8 partitions
        ixw = np.tile(ix_pad.reshape(8 * T, 16).T, (8, 1)).copy()
        im = {
            "xr": Xr,
            "rloc": rl_pad.reshape(T, P).T.copy(),
            "smw": sm_pad.reshape(T, P).T.copy(),
            "idxd": ixw,
            "cntd": np.tile(cnts[None, :], (128, 1)).copy(),
            "iotaf": iotaf,
        }
        in_maps.append(im)
    return T, toff, (nt_lo, nt_hi, sk_lo, sk_hi), in_maps


def kernel(X, u_val, u_rows, u_cols, mu, sigma, u_shape=None, **_kw):
    X = np.asarray(X, dtype=np.float32)
    u_val = np.asarray(u_val, dtype=np.float32)
    mu = np.asarray(mu, dtype=np.float32)
    sigma = np.asarray(sigma, dtype=np.float32)

    T, toff, ntl, in_maps = _prep(X, u_val, u_rows, u_cols, mu, sigma)

    key = (T,) + tuple(tuple(int(x) for x in a) for a in ntl)
    if key not in _cache:
        from concourse.bass_utils import run_bass_kernel_spmd
        nc = _build(T, toff, ntl)
        _cache[key] = (nc, run_bass_kernel_spmd)
    nc, run_bass_kernel_spmd = _cache[key]

    res = run_bass_kernel_spmd(nc, in_maps, core_ids=list(range(M)))
    parts = [res.results[c]["out"].reshape(NS, B, F).transpose(1, 0, 2)
             for c in range(M)]
    return np.ascontiguousarray(np.concatenate(parts, axis=1))



# revision 4
# speedup vs baseline: 2.1250x; 2.1250x over previous
"""Gaussian-weighted GNN message passing on 8 Trainium2 NeuronCores (v7).

out[b,i,f] = sum_{e: row_e=i} softmax_row(w)_e * X[b, col_e, f]
w_e = -0.5 * sum_d (u_val[e,d]-mu[d])^2 / (sigma[d]^2+eps)

Structure (one SPMD program, 8 cores; host does exact softmax + layout):
- Edges sorted by destination row; rows sharded 6250/core; per 128-row
  block, edges split into two packed streams by source column
  (lo: col<32768, hi: rest) to fit int16 gather indices. Static
  per-block tile counts (max across cores); pad slots carry index 0
  with sm=0 so gathers need no per-core counts and no memsets.
- Gathers run in CT-tile chunks (CT*128 indices) rotating over the 4
  SWDGE queues.
- S-matrix construction avoids every per-partition AP-scalar read on
  DVE/ACT (those grab the shared DVE/GpSimd SBUF port pair and starve
  SWDGE descriptor generation, stalling the gather DMAs - measured
  +130us). Instead, for each group of SK tiles, ONE PE matmul with 17
  contraction rows computes
    x[p, c] = sm[p,k] - (q_c-q_pk)^2 - (r_c-r_pk)^2,
  where rloc = 11*q + r (all terms exact in bf16; 7-bit mantissa), and ONE DVE
  relu-with-immediate turns x into S = sm * one_hot(rloc) for SK tiles.
- Per 128-edge tile: accumulate pout += S_tile^T @ Xg in PSUM (bf16
  matmul, f32 acc). PSUM->SBUF on the scalar engine, store via HWDGE.
"""
import os
import numpy as np

B, N, F, E, D = 2, 50000, 128, 800000, 4
BF = B * F
M = 8            # cores
NS = N // M      # rows per core
P = 128          # partitions / tile edge count
NBLK = (NS + P - 1) // P  # 49 row blocks per core
SPLIT = 32768    # int16 gather-index boundary

CT = int(os.environ.get("V7_CT", "4"))      # tiles per gather chunk
GBUFS = int(os.environ.get("V7_GBUFS", "16"))
LOOKC = int(os.environ.get("V7_LOOKC", "12"))
SK = 4           # tiles per S-group (PSUM x tile = [P, SK*128] f32)
SBUFS = int(os.environ.get("V7_SBUFS", "6"))   # S-group buffers
SLOOK = int(os.environ.get("V7_SLOOK", "2"))   # S-group lookahead
NBAND = 3        # wdat bands (17 rows, 32-aligned) per 128 partitions

_cache = {}


def _build(ntlo, nthi, repeat=1):
    from concourse import bacc, mybir
    from concourse.tile import TileContext

    f32 = mybir.dt.float32
    bf16 = mybir.dt.bfloat16
    i16 = mybir.dt.int16

    ntlo = [int(x) for x in ntlo]
    nthi = [int(x) for x in nthi]
    TL, TH = sum(ntlo), sum(nthi)
    TLp = (TL + SK - 1) // SK * SK   # L cols padded to S-group boundary
    THp = (TH + SK - 1) // SK * SK
    T = TLp + THp
    NG = T // SK
    lbase = np.concatenate([[0], np.cumsum(ntlo)]).astype(int)
    hbase = np.concatenate([[0], np.cumsum(nthi)]).astype(int)
    NCL = (TL + CT - 1) // CT
    NCH = (TH + CT - 1) // CT
    WCOL = (NG + NBAND - 1) // NBAND  # wdat column blocks

    nc = bacc.Bacc("TRN2", target_bir_lowering=False, debug=False,
                   num_devices=M, num_swdge_queues=4)
    xr = nc.dram_tensor("xr", [N, BF], bf16, kind="ExternalInput").ap()
    wdat = nc.dram_tensor("wdat", [P, WCOL * P], bf16,
                          kind="ExternalInput").ap()
    rcon = nc.dram_tensor("rcon", [P, SK * P], bf16,
                          kind="ExternalInput").ap()
    idxl = nc.dram_tensor("idxl", [P, 8 * TL], i16, kind="ExternalInput").ap()
    idxh = nc.dram_tensor("idxh", [P, 8 * TH], i16, kind="ExternalInput").ap()
    if repeat == "reg":
        repd = nc.dram_tensor("repd", [1, 1], mybir.dt.int32,
                              kind="ExternalInput").ap()
    out = nc.dram_tensor("out", [NS, BF], f32, kind="ExternalOutput").ap()

    with TileContext(nc) as tc:
        with (
            tc.tile_pool(name="const", bufs=1) as cpool,
            tc.tile_pool(name="g", bufs=2 * GBUFS) as gp,
            tc.tile_pool(name="sw", bufs=SBUFS) as swpool,
            tc.tile_pool(name="px", bufs=2, space="PSUM") as pxpool,
            tc.tile_pool(name="po", bufs=4, space="PSUM") as ppool,
            tc.tile_pool(name="o", bufs=4) as opool,
        ):
            wdat_t = cpool.tile([P, WCOL * P], bf16, tag="wdat")
            nc.sync.dma_start(out=wdat_t[:], in_=wdat[:])
            rcon_t = cpool.tile([P, SK * P], bf16, tag="rcon")
            nc.sync.dma_start(out=rcon_t[:], in_=rcon[:])
            idxl_t = cpool.tile([P, 8 * TL], i16, tag="idxl")
            nc.sync.dma_start(out=idxl_t[:], in_=idxl[:])
            idxh_t = cpool.tile([P, 8 * TH], i16, tag="idxh")
            nc.sync.dma_start(out=idxh_t[:], in_=idxh[:])
            if repeat == "reg":
                rep_t = cpool.tile([1, 1], mybir.dt.int32, tag="rep")
                nc.sync.dma_start(out=rep_t[:], in_=repd[:])
                rep_reg = nc.alloc_registers("rep_reg")
                nc.regs_load(rep_reg, rep_t[0:1, 0:1])

            qctr = [0]

            def block_loop(_iv=None):
                gl, gh = {}, {}
                sg = {}

                def issue(stream, j):
                    idx_t, src, nchunk, tt = (
                        (idxl_t, xr[:SPLIT, :], NCL, TL)
                        if stream == 0
                        else (idxh_t, xr[SPLIT:, :], NCH, TH))
                    store = gl if stream == 0 else gh
                    if j >= nchunk or j in store:
                        return
                    ct = min(CT, tt - j * CT)
                    g = gp.tile([P, CT, BF], bf16, tag="g")
                    nc.gpsimd.dma_gather(
                        out_ap=g[:, :ct, :], in_ap=src,
                        idxs_ap=idx_t[:, 8 * CT * j : 8 * (CT * j + ct)],
                        num_idxs=ct * P, num_idxs_reg=ct * P,
                        elem_size=BF, single_packet=False,
                        queue_num=qctr[0] % 4,
                    )
                    qctr[0] += 1
                    store[j] = g

                def sbuild(grp):
                    if grp >= NG or grp in sg:
                        return
                    band = grp % NBAND
                    colb = grp // NBAND
                    xw = pxpool.tile([P, SK * P], f32, tag="xw",
                                     space="PSUM")
                    nc.tensor.matmul(
                        out=xw[:],
                        lhsT=wdat_t[32 * band : 32 * band + 17,
                                    colb * P : (colb + 1) * P],
                        rhs=rcon_t[32 * band : 32 * band + 17, :],
                        start=True, stop=True, skip_group_check=True)
                    sw = swpool.tile([P, SK * P], bf16, tag="sw")
                    nc.vector.tensor_scalar(
                        out=sw[:], in0=xw[:],
                        scalar1=0.0, scalar2=None,
                        op0=mybir.AluOpType.max,
                    )
                    sg[grp] = sw

                for j in range(LOOKC):
                    issue(0, j)
                    issue(1, j)
                for gidx in range(SLOOK + 1):
                    sbuild(gidx)
                    sbuild(TLp // SK + gidx)

                jl = jh = 0
                sgl = [0, TLp // SK]  # next-unbuilt S-group per stream

                for b in range(NBLK):
                    r0 = b * P
                    rows_here = min(P, NS - r0)
                    nt = ntlo[b] + nthi[b]
                    pout = ppool.tile([P, BF], f32, tag="pout", space="PSUM")
                    k = 0
                    for half in (0, 1):
                        nth = (ntlo, nthi)[half][b]
                        base = (lbase, hbase)[half][b]
                        off = 0 if half == 0 else TLp
                        store = (gl, gh)[half]
                        for kk in range(nth):
                            t = base + kk
                            j = t // CT
                            if half == 0 and j > jl:
                                jl = j
                                issue(0, j + LOOKC - 1)
                            if half == 1 and j > jh:
                                jh = j
                                issue(1, j + LOOKC - 1)
                            col = off + t
                            grp = col // SK
                            if grp > sgl[half]:
                                sgl[half] = grp
                                sbuild(grp + SLOOK)
                            sbuild(grp)
                            sw = sg[grp]
                            g = store[j]
                            sl = col % SK
                            nc.tensor.matmul(
                                out=pout[:],
                                lhsT=sw[:, sl * P : (sl + 1) * P],
                                rhs=g[:, t % CT, :],
                                start=(k == 0), stop=(k == nt - 1),
                                skip_group_check=True)
                            k += 1
                    osb = opool.tile([P, BF], f32, tag="osb")
                    nc.scalar.activation(
                        out=osb[:], in_=pout[:],
                        func=mybir.ActivationFunctionType.Copy,
                    )
                    nc.sync.dma_start(out=out[r0 : r0 + rows_here, :],
                                      in_=osb[:rows_here, :])
                gl.clear()
                gh.clear()
                sg.clear()

            if repeat == 1:
                block_loop()
            elif repeat == "reg":
                with tc.For_i(0, rep_reg, 1) as _i:
                    block_loop(_i)
            else:
                with tc.For_i(0, repeat, 1) as _i:
                    block_loop(_i)

    nc.compile()
    return nc


def _prep(X, u_val, u_rows, u_cols, mu, sigma):
    """Host-side shard/sort/pad + exact softmax weights + wdat/rcon."""
    import ml_dtypes
    bf16 = ml_dtypes.bfloat16

    rows = np.asarray(u_rows).astype(np.int64)
    u = np.asarray(u_val).astype(np.float64)
    muf = np.asarray(mu)[0].astype(np.float64)
    sgf = np.asarray(sigma)[0].astype(np.float64)
    w = -0.5 * np.sum((u - muf) ** 2 / (sgf**2 + 1e-14), axis=1)
    z = np.exp(w)  # float64: no underflow for this data distribution
    ssum = np.bincount(rows, weights=z, minlength=N)
    sm = (z / ssum[rows]).astype(np.float32)

    cols = np.asarray(u_cols).astype(np.int64)
    perm = np.lexsort((cols, rows))
    rows_s, cols_s, sm_s = rows[perm], cols[perm], sm[perm]

    core_of = rows_s // NS
    blk_of = (rows_s % NS) // P
    is_hi = (cols_s >= SPLIT).astype(np.int64)
    cnt = np.zeros((2, M, NBLK), dtype=np.int64)
    np.add.at(cnt, (is_hi, core_of, blk_of), 1)
    ntlo = np.maximum(1, (cnt[0].max(axis=0) + P - 1) // P).astype(np.int64)
    nthi = np.maximum(1, (cnt[1].max(axis=0) + P - 1) // P).astype(np.int64)
    TL, TH = int(ntlo.sum()), int(nthi.sum())
    TLp = (TL + SK - 1) // SK * SK
    THp = (TH + SK - 1) // SK * SK
    T = TLp + THp
    NG = T // SK
    WCOL = (NG + NBAND - 1) // NBAND
    lbase = np.concatenate([[0], np.cumsum(ntlo)]).astype(np.int64)
    hbase = np.concatenate([[0], np.cumsum(nthi)]).astype(np.int64)

    Xr = np.ascontiguousarray(
        np.asarray(X).transpose(1, 0, 2).reshape(N, BF)).astype(bf16)

    # rcon [128, SK*128] bf16: bands of 17 rows (same content per band)
    iota = np.arange(P)
    hi_c, lo_c = iota // 11, iota % 11
    rband = np.zeros((17, SK * P), dtype=np.float32)
    rband[0, :] = np.tile(-1.0 * (hi_c**2 + lo_c**2), SK)
    for kk in range(SK):
        s = kk * P
        rband[1 + 4 * kk, s : s + P] = hi_c
        rband[2 + 4 * kk, s : s + P] = lo_c
        rband[3 + 4 * kk, s : s + P] = 1.0
        rband[4 + 4 * kk, s : s + P] = 1.0
    rcon = np.zeros((P, SK * P), dtype=np.float32)
    for bnd in range(NBAND):
        rcon[32 * bnd : 32 * bnd + 17, :] = rband
    rcon = rcon.astype(bf16)

    def wrap_idx(ix):
        tt = ix.shape[0] // 128
        return np.tile(ix.reshape(8 * tt, 16).T, (8, 1)).copy()

    in_maps = []
    for c in range(M):
        # per-(col,slot) rloc (int, 0 for pads) and sm (0 for pads)
        rl = np.zeros((T, P), dtype=np.float32)
        smv = np.zeros((T, P), dtype=np.float32)
        ixl = np.zeros((TL * P,), dtype=np.int16)
        ixh = np.zeros((TH * P,), dtype=np.int16)
        lo = np.searchsorted(rows_s, c * NS)
        hi = np.searchsorted(rows_s, (c + 1) * NS)
        e_cols = cols_s[lo:hi]
        e_sm = sm_s[lo:hi]
        e_rows = rows_s[lo:hi]
        e_blk = (e_rows - c * NS) // P
        e_rl = ((e_rows - c * NS) % P).astype(np.float32)
        e_hi = e_cols >= SPLIT
        order = np.lexsort((e_cols, e_hi, e_blk))
        e_cols, e_sm, e_rl = e_cols[order], e_sm[order], e_rl[order]
        e_blk, e_hi = e_blk[order], e_hi[order]
        for b in range(NBLK):
            for half, ix_arr, base_arr, bias, c_off in (
                (0, ixl, lbase, 0, 0),
                (1, ixh, hbase, SPLIT, TLp),
            ):
                mask = (e_blk == b) & (e_hi == bool(half))
                n_e = int(mask.sum())
                if n_e == 0:
                    continue
                s0 = int(base_arr[b]) * P
                ix_arr[s0 : s0 + n_e] = (e_cols[mask] - bias).astype(np.int16)
                col0 = c_off + int(base_arr[b])
                fl = np.arange(n_e)
                rl[col0 + fl // P, fl % P] = e_rl[mask]
                smv[col0 + fl // P, fl % P] = e_sm[mask]
        # wdat [128, WCOL*128] bf16: group g -> band g%NBAND, colblk g//NBAND
        wd = np.zeros((P, WCOL * P), dtype=np.float32)
        hi_r, lo_r = rl // 11, rl % 11
        for g in range(NG):
            bnd, cb = g % NBAND, g // NBAND
            rowsl = slice(32 * bnd, 32 * bnd + 17)
            colsl = slice(cb * P, (cb + 1) * P)
            blk17 = np.zeros((17, P), dtype=np.float32)
            blk17[0, :] = 1.0
            for kk in range(SK):
                tcol = g * SK + kk
                blk17[1 + 4 * kk] = 2.0 * hi_r[tcol]
                blk17[2 + 4 * kk] = 2.0 * lo_r[tcol]
                blk17[3 + 4 * kk] = smv[tcol]
                blk17[4 + 4 * kk] = -1.0 * (hi_r[tcol] ** 2 + lo_r[tcol] ** 2)
            wd[rowsl, colsl] += blk17
        im = {
            "xr": Xr,
            "wdat": wd.astype(bf16),
            "rcon": rcon,
            "idxl": wrap_idx(ixl),
            "idxh": wrap_idx(ixh),
        }
        in_maps.append(im)
    return ntlo, nthi, in_maps


def kernel(X, u_val, u_rows, u_cols, mu, sigma, u_shape=None, **_kw):
    X = np.asarray(X, dtype=np.float32)
    u_val = np.asarray(u_val, dtype=np.float32)
    mu = np.asarray(mu, dtype=np.float32)
    sigma = np.asarray(sigma, dtype=np.float32)

    ntlo, nthi, in_maps = _prep(X, u_val, u_rows, u_cols, mu, sigma)

    key = tuple(int(x) for x in ntlo) + tuple(int(x) for x in nthi)
    if key not in _cache:
        from concourse.bass_utils import run_bass_kernel_spmd
        nc = _build(ntlo, nthi)
        _cache[key] = (nc, run_bass_kernel_spmd)
    nc, run_bass_kernel_spmd = _cache[key]

    res = run_bass_kernel_spmd(nc, in_maps, core_ids=list(range(M)))
    parts = [res.results[c]["out"].reshape(NS, B, F).transpose(1, 0, 2)
             for c in range(M)]
    return np.ascontiguousarray(np.concatenate(parts, axis=1))
